# revision 15
# baseline (speedup 1.0000x reference)
"""Trainium2 Bass kernel for nn_BondGraphNetDecoder.

Math (per head with tuple size k, edge list M, latent z of batch B):
  g[m]   = concat_j template[idx[m, j]]            (batch-independent)
  x[b,m] = concat(g[m], z[b])
  h      = elu(x @ Win + bin); h = elu(h @ Whid_i + bhid_i) (i=0,1)
  out    = h @ Wout + bout

Factorization used on device:
  pre0[m]    = sum_j Win_j.T-contracted gathered template rows (batch-indep)
  bias1[b]   = Win_z.T @ z[b] + bin                (tiny, per batch)
  layer1     = elu(pre0 + bias1[b]);  hidden layers elu(h @ W + c)
  elu(s) exact identity: elu(s) = max(s, min(exp(s), 1) - 1)

All activations kept feature-major ([H=128 partitions, edges free]) so the
hidden-layer matmuls chain with no transposes; only the gathered template
rows are transposed (PE transpose) once per 128-edge block.

Sharding: edge lists split evenly across the 8 cores; weights/z/template
replicated. Outputs concatenated on host; bout added on host (scalar).
"""

import sys

import numpy as np

for _p in ("/opt/trn_rl_repo", "/root/.axon_site/_ro/trn_rl_repo"):
    if _p not in sys.path:
        sys.path.append(_p)

import concourse.bass as bass
import concourse.mybir as mybir
import concourse.tile as tile
from concourse import bacc
from concourse.bass_utils import run_bass_kernel_spmd
from concourse.masks import make_identity

FP = mybir.dt.float32
FPR = mybir.dt.float32r
I32 = mybir.dt.int32
AF = mybir.ActivationFunctionType
OP = mybir.AluOpType

B, L, H, N = 16, 128, 128, 4096
NB, NA, ND = 4096, 8192, 8192
NCORES = 8
NBc, NAc, NDc = NB // NCORES, NA // NCORES, ND // NCORES
P = 128
TW = 512  # edges per compute tile

HEADS = ["bond", "angle", "dcos", "dsin"]
# gather groups: (group_name, head_names, k, edges_per_core, index_input_name)
GROUPS = [
    ("bond", ["bond"], 2, NBc, "bond_index"),
    ("angle", ["angle"], 3, NAc, "angle_index"),
    ("torsion", ["dcos", "dsin"], 4, NDc, "torsion_index"),
]
HEAD_K = {"bond": 2, "angle": 3, "dcos": 4, "dsin": 4}
HEAD_M = {"bond": NBc, "angle": NAc, "dcos": NDc, "dsin": NDc}

_CACHE = {}


def _emit_elu_stage(nc, apool, ypool, srcs_and_biases):
    """Stage-grouped ELU over several (src, bias) pairs.

    y = elu(src + bias) = max(src + bias, min(exp(src + bias), 1) - 1).
    Emits all Exp ops (ACT), then all min/add ops (GPSIMD), then all
    combine ops (DVE) so the three engines pipeline across the group.
    src may be PSUM or SBUF [128, TW]; bias is [128, 1] SBUF per-partition.
    """
    es = []
    for src_ap, bias_ap in srcs_and_biases:
        e = apool.tile([P, TW], FP, tag="e")
        nc.scalar.activation(out=e[:], in_=src_ap, func=AF.Exp, bias=bias_ap, scale=1.0)
        es.append(e)
    tts = []
    for i, e in enumerate(es):
        t = apool.tile([P, TW], FP, tag="t")
        eng = nc.gpsimd if i % 2 == 0 else nc.vector
        eng.tensor_scalar(
            out=t[:], in0=e[:], scalar1=1.0, scalar2=-1.0, op0=OP.min, op1=OP.add
        )
        tts.append(t)
    ys = []
    for (src_ap, bias_ap), t in zip(srcs_and_biases, tts):
        y = ypool.tile([P, TW], FPR, tag="y")
        nc.vector.scalar_tensor_tensor(
            out=y[:], in0=src_ap, scalar=bias_ap, in1=t[:], op0=OP.add, op1=OP.max
        )
        ys.append(y)
    return ys


def _build():
    nc = bacc.Bacc("TRN2", target_bir_lowering=False, debug=False)

    z_d = nc.dram_tensor("z", [B, L], FP, kind="ExternalInput")
    tmpl_d = nc.dram_tensor("template_node_repr", [N, H], FP, kind="ExternalInput")
    idx_d = {
        "bond_index": nc.dram_tensor("bond_index", [NBc, 2], I32, kind="ExternalInput"),
        "angle_index": nc.dram_tensor("angle_index", [NAc, 3], I32, kind="ExternalInput"),
        "torsion_index": nc.dram_tensor(
            "torsion_index", [NDc, 4], I32, kind="ExternalInput"
        ),
    }
    wdram = {}
    for name in HEADS:
        k = HEAD_K[name]
        wdram[name, "Win"] = nc.dram_tensor(
            f"{name}_Win", [k * H + L, H], FP, kind="ExternalInput"
        )
        wdram[name, "bin"] = nc.dram_tensor(f"{name}_bin", [H, 1], FP, kind="ExternalInput")
        wdram[name, "Whid"] = nc.dram_tensor(
            f"{name}_Whid", [2, H, H], FP, kind="ExternalInput"
        )
        wdram[name, "bhid"] = nc.dram_tensor(
            f"{name}_bhid", [2, H, 1], FP, kind="ExternalInput"
        )
        wdram[name, "Wout"] = nc.dram_tensor(f"{name}_Wout", [H, 1], FP, kind="ExternalInput")
    out_d = {
        name: nc.dram_tensor(f"{name}_out", [B, HEAD_M[name]], FP, kind="ExternalOutput")
        for name in HEADS
    }

    with tile.TileContext(nc) as tc:
        with (
            tc.tile_pool(name="const", bufs=1) as constp,
            tc.tile_pool(name="wpool", bufs=1) as wpool,
            tc.tile_pool(name="gath", bufs=6) as gpool,
            tc.tile_pool(name="gt", bufs=2) as gtpool,
            tc.tile_pool(name="pre", bufs=3) as prepool,
            tc.tile_pool(name="acts", bufs=6) as apool,
            tc.tile_pool(name="ys", bufs=8) as ypool,
            tc.tile_pool(name="ostage", bufs=2) as opool,
            tc.tile_pool(name="ps_tr", bufs=1, space="PSUM") as ps_tr,
            tc.tile_pool(name="ps_pre", bufs=1, space="PSUM") as ps_pre,
            tc.tile_pool(name="ps_mm", bufs=4, space="PSUM") as ps_mm,
            tc.tile_pool(name="ps_out", bufs=2, space="PSUM") as ps_out,
        ):
            ident = constp.tile([P, P], FP)
            make_identity(nc, ident[:])

            # z transposed: [L, B] (element-strided DMA; one-time, tiny)
            zT = constp.tile([P, B], FP)
            nc.sync.dma_start(out=zT[:], in_=z_d[:].rearrange("b l -> l b"))

            W = {}  # (name, kind, j) -> sbuf tile
            bias1 = {}
            for name in HEADS:
                k = HEAD_K[name]
                for j in range(k):
                    stg = gpool.tile([P, H], FP, tag="wstg")
                    nc.sync.dma_start(out=stg[:], in_=wdram[name, "Win"][j * P : (j + 1) * P, :])
                    t_ = wpool.tile([P, H], FPR, tag=f"wi_{name}{j}")
                    nc.vector.tensor_copy(out=t_[:], in_=stg[:])
                    W[name, "wi", j] = t_
                wz = wpool.tile([P, H], FP, tag=f"wz_{name}")
                nc.sync.dma_start(out=wz[:], in_=wdram[name, "Win"][k * P : k * P + P, :])
                W[name, "wz", 0] = wz
                for i in range(2):
                    stg = gpool.tile([P, H], FP, tag="wstg")
                    nc.sync.dma_start(out=stg[:], in_=wdram[name, "Whid"][:][i : i + 1].rearrange("a p h -> (a p) h"))
                    wh = wpool.tile([P, H], FPR, tag=f"wh{i}_{name}")
                    nc.vector.tensor_copy(out=wh[:], in_=stg[:])
                    W[name, "wh", i] = wh
                    bh = wpool.tile([P, 1], FP, tag=f"bh{i}_{name}")
                    nc.sync.dma_start(out=bh[:], in_=wdram[name, "bhid"][:][i : i + 1].rearrange("a p o -> (a p) o"))
                    W[name, "bh", i] = bh
                wo = wpool.tile([P, 1], FP, tag=f"wo_{name}")
                nc.sync.dma_start(out=wo[:], in_=wdram[name, "Wout"][:])
                W[name, "wo", 0] = wo
                binv = wpool.tile([P, 1], FP, tag=f"bin_{name}")
                nc.sync.dma_start(out=binv[:], in_=wdram[name, "bin"][:])

                # bias1[name] = Win_z.T @ zT + bin  -> [H, B]
                zw_ps = ps_pre.tile([P, B], FP, tag="pre0")
                nc.tensor.matmul(out=zw_ps[:], lhsT=wz[:], rhs=zT[:], start=True, stop=True)
                b1 = wpool.tile([P, B], FP, tag=f"b1_{name}")
                nc.vector.tensor_scalar(
                    out=b1[:], in0=zw_ps[:], scalar1=binv[:], scalar2=None, op0=OP.add
                )
                bias1[name] = b1

                # expanded Wout variants: [128, 16] with Wout in column b
                for b in range(B):
                    we = wpool.tile([P, B], FPR, tag=f"we_{name}{b}")
                    nc.vector.memset(we[:].bitcast(FP), 0.0)
                    nc.vector.tensor_copy(out=we[:, b : b + 1], in_=wo[:])
                    W[name, "we", b] = we

            for gname, heads, k, M, idxname in GROUPS:
                for ti in range(M // TW):
                    # indices for this tile, one [128, k] block per 128 edges
                    idxts = []
                    for blk in range(TW // P):
                        it = gpool.tile([P, k], I32, tag="idx")
                        r0 = ti * TW + blk * P
                        nc.sync.dma_start(out=it[:], in_=idx_d[idxname][r0 : r0 + P, :])
                        idxts.append(it)
                    # gather + transpose -> gT_j [H, TW] feature-major
                    gTs = []
                    for j in range(k):
                        gT = gtpool.tile([P, TW], FPR, tag=f"gt{j}")
                        for blk in range(TW // P):
                            g = gpool.tile([P, P], FP, tag="graw")
                            nc.gpsimd.indirect_dma_start(
                                out=g[:],
                                out_offset=None,
                                in_=tmpl_d[:],
                                in_offset=bass.IndirectOffsetOnAxis(
                                    ap=idxts[blk][:, j : j + 1], axis=0
                                ),
                            )
                            pt = ps_tr.tile([P, P], FP, tag="ptr")
                            nc.tensor.transpose(out=pt[:], in_=g[:], identity=ident[:])
                            nc.scalar.copy(out=gT[:, blk * P : (blk + 1) * P], in_=pt[:])
                        gTs.append(gT)

                    for name in heads:
                        # first layer, batch-independent part: pre0 = sum_j Win_j.T @ gT_j
                        pre_ps = ps_pre.tile([P, TW], FP, tag="pre0")
                        for j in range(k):
                            nc.tensor.matmul(
                                out=pre_ps[:],
                                lhsT=W[name, "wi", j][:],
                                rhs=gTs[j][:],
                                start=(j == 0),
                                stop=(j == k - 1),
                            )
                        pre = prepool.tile([P, TW], FP, tag="pre")
                        nc.scalar.copy(out=pre[:], in_=pre_ps[:])

                        po = ps_out.tile([B, TW], FP, tag="po")
                        G = 4  # batch group size for stage pipelining
                        for g0 in range(0, B, G):
                            bs = list(range(g0, min(g0 + G, B)))
                            y0s = _emit_elu_stage(
                                nc, apool, ypool,
                                [(pre[:], bias1[name][:, b : b + 1]) for b in bs],
                            )
                            mm1s = []
                            for y0 in y0s:
                                mm1 = ps_mm.tile([P, TW], FP, tag="mm")
                                nc.tensor.matmul(
                                    out=mm1[:], lhsT=W[name, "wh", 0][:],
                                    rhs=y0[:],
                                    start=True, stop=True,
                                )
                                mm1s.append(mm1)
                            y1s = _emit_elu_stage(
                                nc, apool, ypool,
                                [(mm1[:], W[name, "bh", 0][:]) for mm1 in mm1s],
                            )
                            mm2s = []
                            for y1 in y1s:
                                mm2 = ps_mm.tile([P, TW], FP, tag="mm")
                                nc.tensor.matmul(
                                    out=mm2[:], lhsT=W[name, "wh", 1][:],
                                    rhs=y1[:],
                                    start=True, stop=True,
                                )
                                mm2s.append(mm2)
                            y2s = _emit_elu_stage(
                                nc, apool, ypool,
                                [(mm2[:], W[name, "bh", 1][:]) for mm2 in mm2s],
                            )
                            for b, y2 in zip(bs, y2s):
                                # out row b accumulates into partition b of po
                                nc.tensor.matmul(
                                    out=po[:], lhsT=W[name, "we", b][:],
                                    rhs=y2[:],
                                    start=(b == 0), stop=(b == B - 1),
                                    skip_group_check=True,
                                )
                        stage = opool.tile([B, TW], FP, tag="ost")
                        nc.scalar.copy(out=stage[:], in_=po[:])
                        nc.sync.dma_start(
                            out=out_d[name][:, ti * TW : (ti + 1) * TW], in_=stage[:]
                        )
    nc.compile()
    return nc


def _get_nc():
    if "nc" not in _CACHE:
        _CACHE["nc"] = _build()
    return _CACHE["nc"]


def kernel(**inputs):
    nc = _get_nc()
    f32 = np.float32

    def arr(x, dt=f32):
        return np.ascontiguousarray(np.asarray(x), dtype=dt)

    base = {
        "z": arr(inputs["z"]),
        "template_node_repr": arr(inputs["template_node_repr"]),
    }
    for name in HEADS:
        k = HEAD_K[name]
        base[f"{name}_Win"] = arr(inputs[f"{name}_Win"])
        base[f"{name}_bin"] = arr(inputs[f"{name}_bin"]).reshape(H, 1)
        base[f"{name}_Whid"] = arr(inputs[f"{name}_Whid"])
        base[f"{name}_bhid"] = arr(inputs[f"{name}_bhid"]).reshape(2, H, 1)
        base[f"{name}_Wout"] = arr(inputs[f"{name}_Wout"]).reshape(H, 1)

    bond_idx = arr(inputs["bond_index"], np.int32)
    angle_idx = arr(inputs["angle_index"], np.int32)
    tors_idx = arr(inputs["torsion_index"], np.int32)

    in_maps = []
    for c in range(NCORES):
        m = dict(base)
        m["bond_index"] = np.ascontiguousarray(bond_idx[c * NBc : (c + 1) * NBc])
        m["angle_index"] = np.ascontiguousarray(angle_idx[c * NAc : (c + 1) * NAc])
        m["torsion_index"] = np.ascontiguousarray(tors_idx[c * NDc : (c + 1) * NDc])
        in_maps.append(m)

    res = run_bass_kernel_spmd(nc, in_maps, core_ids=list(range(NCORES)))
    outs = res.results

    def cat(name):
        full = np.concatenate([outs[c][f"{name}_out"] for c in range(NCORES)], axis=1)
        bout = float(np.asarray(inputs[f"{name}_bout"]).reshape(-1)[0])
        return (full + f32(bout)).astype(f32)

    return cat("bond"), cat("angle"), cat("dcos"), cat("dsin")


# revision 16
# speedup vs baseline: 5.0740x; 5.0740x over previous
"""Trainium2 Bass kernel for nn_BondGraphNetDecoder.

Math (per head with tuple size k, edge list M, latent z of batch B):
  g[m]   = concat_j template[idx[m, j]]            (batch-independent)
  x[b,m] = concat(g[m], z[b])
  h      = elu(x @ Win + bin); h = elu(h @ Whid_i + bhid_i) (i=0,1)
  out    = h @ Wout + bout

Factorization used on device:
  pre0[m]    = sum_j Win_j.T-contracted gathered template rows (batch-indep)
  bias1[b]   = Win_z.T @ z[b] + bin                (tiny, per batch)
  layer1     = elu(pre0 + bias1[b]);  hidden layers elu(h @ W + c)
  elu(s) exact identity: elu(s) = max(s, min(exp(s), 1) - 1)

All activations kept feature-major ([H=128 partitions, edges free]) so the
hidden-layer matmuls chain with no transposes; only the gathered template
rows are transposed (PE transpose) once per 128-edge block.

Sharding: edge lists split evenly across the 8 cores; weights/z/template
replicated. Outputs concatenated on host; bout added on host (scalar).
"""

import sys

import numpy as np

for _p in ("/opt/trn_rl_repo", "/root/.axon_site/_ro/trn_rl_repo"):
    if _p not in sys.path:
        sys.path.append(_p)

import concourse.bass as bass
import concourse.mybir as mybir
import concourse.tile as tile
from concourse import bacc
from concourse.bass_utils import run_bass_kernel_spmd
from concourse.masks import make_identity

FP = mybir.dt.float32
FPR = mybir.dt.float32r
I32 = mybir.dt.int32
AF = mybir.ActivationFunctionType
OP = mybir.AluOpType

B, L, H, N = 16, 128, 128, 4096
NB, NA, ND = 4096, 8192, 8192
NCORES = 8
NBc, NAc, NDc = NB // NCORES, NA // NCORES, ND // NCORES
P = 128
TW = 512  # edges per compute tile

HEADS = ["bond", "angle", "dcos", "dsin"]
# gather groups: (group_name, head_names, k, edges_per_core, index_input_name)
GROUPS = [
    ("bond", ["bond"], 2, NBc, "bond_index"),
    ("angle", ["angle"], 3, NAc, "angle_index"),
    ("torsion", ["dcos", "dsin"], 4, NDc, "torsion_index"),
]
HEAD_K = {"bond": 2, "angle": 3, "dcos": 4, "dsin": 4}
HEAD_M = {"bond": NBc, "angle": NAc, "dcos": NDc, "dsin": NDc}

_CACHE = {}


def _emit_elu_stage(nc, apool, ypool, srcs_and_biases, t_mode="alt"):
    """Stage-grouped ELU over several (src, bias) pairs.

    y = elu(src + bias) = max(src + bias, min(exp(src + bias), 1) - 1).
    Emits all Exp ops (ACT), then all min/add ops (GPSIMD), then all
    combine ops (DVE) so the three engines pipeline across the group.
    src may be PSUM or SBUF [128, TW]; bias is [128, 1] SBUF per-partition.
    """
    es = []
    for src_ap, bias_ap in srcs_and_biases:
        e = apool.tile([P, TW], FP, tag="e")
        nc.scalar.activation(out=e[:], in_=src_ap, func=AF.Exp, bias=bias_ap, scale=1.0)
        es.append(e)
    tts = []
    for i, e in enumerate(es):
        t = apool.tile([P, TW], FP, tag="t")
        if t_mode == "alt":
            eng = nc.gpsimd if i % 2 == 0 else nc.vector
        elif t_mode == "dve":
            eng = nc.vector
        else:
            eng = nc.gpsimd
        eng.tensor_scalar(
            out=t[:], in0=e[:], scalar1=1.0, scalar2=-1.0, op0=OP.min, op1=OP.add
        )
        tts.append(t)
    ys = []
    for (src_ap, bias_ap), t in zip(srcs_and_biases, tts):
        y = ypool.tile([P, TW], FPR, tag="y")
        nc.vector.scalar_tensor_tensor(
            out=y[:], in0=src_ap, scalar=bias_ap, in1=t[:], op0=OP.add, op1=OP.max
        )
        ys.append(y)
    return ys


def _build(reps=1, t_mode="alt"):
    nc = bacc.Bacc("TRN2", target_bir_lowering=False, debug=False)

    z_d = nc.dram_tensor("z", [B, L], FP, kind="ExternalInput")
    tmpl_d = nc.dram_tensor("template_node_repr", [N, H], FP, kind="ExternalInput")
    idx_d = {
        "bond_index": nc.dram_tensor("bond_index", [NBc, 2], I32, kind="ExternalInput"),
        "angle_index": nc.dram_tensor("angle_index", [NAc, 3], I32, kind="ExternalInput"),
        "torsion_index": nc.dram_tensor(
            "torsion_index", [NDc, 4], I32, kind="ExternalInput"
        ),
    }
    wdram = {}
    for name in HEADS:
        k = HEAD_K[name]
        wdram[name, "Win"] = nc.dram_tensor(
            f"{name}_Win", [k * H + L, H], FP, kind="ExternalInput"
        )
        wdram[name, "bin"] = nc.dram_tensor(f"{name}_bin", [H, 1], FP, kind="ExternalInput")
        wdram[name, "Whid"] = nc.dram_tensor(
            f"{name}_Whid", [2, H, H], FP, kind="ExternalInput"
        )
        wdram[name, "bhid"] = nc.dram_tensor(
            f"{name}_bhid", [2, H, 1], FP, kind="ExternalInput"
        )
        wdram[name, "Wout"] = nc.dram_tensor(f"{name}_Wout", [H, 1], FP, kind="ExternalInput")
    out_d = {
        name: nc.dram_tensor(f"{name}_out", [B, HEAD_M[name]], FP, kind="ExternalOutput")
        for name in HEADS
    }

    with tile.TileContext(nc) as tc:
        with (
            tc.tile_pool(name="const", bufs=1) as constp,
            tc.tile_pool(name="wpool", bufs=1) as wpool,
            tc.tile_pool(name="gath", bufs=6) as gpool,
            tc.tile_pool(name="gt", bufs=2) as gtpool,
            tc.tile_pool(name="pre", bufs=3) as prepool,
            tc.tile_pool(name="acts", bufs=6) as apool,
            tc.tile_pool(name="ys", bufs=8) as ypool,
            tc.tile_pool(name="ostage", bufs=2) as opool,
            tc.tile_pool(name="ps_tr", bufs=1, space="PSUM") as ps_tr,
            tc.tile_pool(name="ps_pre", bufs=1, space="PSUM") as ps_pre,
            tc.tile_pool(name="ps_mm", bufs=4, space="PSUM") as ps_mm,
            tc.tile_pool(name="ps_out", bufs=2, space="PSUM") as ps_out,
        ):
            ident = constp.tile([P, P], FP)
            make_identity(nc, ident[:])

            # z transposed: [L, B] (element-strided DMA; one-time, tiny)
            zT = constp.tile([P, B], FP)
            nc.sync.dma_start(out=zT[:], in_=z_d[:].rearrange("b l -> l b"))

            W = {}  # (name, kind, j) -> sbuf tile
            bias1 = {}
            for name in HEADS:
                k = HEAD_K[name]
                for j in range(k):
                    stg = gpool.tile([P, H], FP, tag="wstg")
                    nc.sync.dma_start(out=stg[:], in_=wdram[name, "Win"][j * P : (j + 1) * P, :])
                    t_ = wpool.tile([P, H], FPR, tag=f"wi_{name}{j}")
                    nc.vector.tensor_copy(out=t_[:], in_=stg[:])
                    W[name, "wi", j] = t_
                wz = wpool.tile([P, H], FP, tag=f"wz_{name}")
                nc.sync.dma_start(out=wz[:], in_=wdram[name, "Win"][k * P : k * P + P, :])
                W[name, "wz", 0] = wz
                for i in range(2):
                    stg = gpool.tile([P, H], FP, tag="wstg")
                    nc.sync.dma_start(out=stg[:], in_=wdram[name, "Whid"][:][i : i + 1].rearrange("a p h -> (a p) h"))
                    wh = wpool.tile([P, H], FPR, tag=f"wh{i}_{name}")
                    nc.vector.tensor_copy(out=wh[:], in_=stg[:])
                    W[name, "wh", i] = wh
                    bh = wpool.tile([P, 1], FP, tag=f"bh{i}_{name}")
                    nc.sync.dma_start(out=bh[:], in_=wdram[name, "bhid"][:][i : i + 1].rearrange("a p o -> (a p) o"))
                    W[name, "bh", i] = bh
                wo = wpool.tile([P, 1], FP, tag=f"wo_{name}")
                nc.sync.dma_start(out=wo[:], in_=wdram[name, "Wout"][:])
                W[name, "wo", 0] = wo
                binv = wpool.tile([P, 1], FP, tag=f"bin_{name}")
                nc.sync.dma_start(out=binv[:], in_=wdram[name, "bin"][:])

                # bias1[name] = Win_z.T @ zT + bin  -> [H, B]
                zw_ps = ps_pre.tile([P, B], FP, tag="pre0")
                nc.tensor.matmul(out=zw_ps[:], lhsT=wz[:], rhs=zT[:], start=True, stop=True)
                b1 = wpool.tile([P, B], FP, tag=f"b1_{name}")
                nc.vector.tensor_scalar(
                    out=b1[:], in0=zw_ps[:], scalar1=binv[:], scalar2=None, op0=OP.add
                )
                bias1[name] = b1

                # expanded Wout variants: [128, 16] with Wout in column b
                for b in range(B):
                    we = wpool.tile([P, B], FPR, tag=f"we_{name}{b}")
                    nc.vector.memset(we[:].bitcast(FP), 0.0)
                    nc.vector.tensor_copy(out=we[:, b : b + 1], in_=wo[:])
                    W[name, "we", b] = we

            for _rep in range(reps):
              for gname, heads, k, M, idxname in GROUPS:
                for ti in range(M // TW):
                    # indices for this tile, one [128, k] block per 128 edges
                    idxts = []
                    for blk in range(TW // P):
                        it = gpool.tile([P, k], I32, tag="idx")
                        r0 = ti * TW + blk * P
                        nc.sync.dma_start(out=it[:], in_=idx_d[idxname][r0 : r0 + P, :])
                        idxts.append(it)
                    # gather + transpose -> gT_j [H, TW] feature-major
                    gTs = []
                    for j in range(k):
                        gT = gtpool.tile([P, TW], FPR, tag=f"gt{j}")
                        for blk in range(TW // P):
                            g = gpool.tile([P, P], FP, tag="graw")
                            nc.gpsimd.indirect_dma_start(
                                out=g[:],
                                out_offset=None,
                                in_=tmpl_d[:],
                                in_offset=bass.IndirectOffsetOnAxis(
                                    ap=idxts[blk][:, j : j + 1], axis=0
                                ),
                            )
                            pt = ps_tr.tile([P, P], FP, tag="ptr")
                            nc.tensor.transpose(out=pt[:], in_=g[:], identity=ident[:])
                            nc.scalar.copy(out=gT[:, blk * P : (blk + 1) * P], in_=pt[:])
                        gTs.append(gT)

                    for name in heads:
                        # first layer, batch-independent part: pre0 = sum_j Win_j.T @ gT_j
                        pre_ps = ps_pre.tile([P, TW], FP, tag="pre0")
                        for j in range(k):
                            nc.tensor.matmul(
                                out=pre_ps[:],
                                lhsT=W[name, "wi", j][:],
                                rhs=gTs[j][:],
                                start=(j == 0),
                                stop=(j == k - 1),
                            )
                        pre = prepool.tile([P, TW], FP, tag="pre")
                        nc.scalar.copy(out=pre[:], in_=pre_ps[:])

                        po = ps_out.tile([B, TW], FP, tag="po")
                        G = 4  # batch group size for stage pipelining
                        for g0 in range(0, B, G):
                            bs = list(range(g0, min(g0 + G, B)))
                            y0s = _emit_elu_stage(
                                nc, apool, ypool,
                                [(pre[:], bias1[name][:, b : b + 1]) for b in bs],
                                t_mode,
                            )
                            mm1s = []
                            for y0 in y0s:
                                mm1 = ps_mm.tile([P, TW], FP, tag="mm")
                                nc.tensor.matmul(
                                    out=mm1[:], lhsT=W[name, "wh", 0][:],
                                    rhs=y0[:],
                                    start=True, stop=True,
                                )
                                mm1s.append(mm1)
                            y1s = _emit_elu_stage(
                                nc, apool, ypool,
                                [(mm1[:], W[name, "bh", 0][:]) for mm1 in mm1s],
                                t_mode,
                            )
                            mm2s = []
                            for y1 in y1s:
                                mm2 = ps_mm.tile([P, TW], FP, tag="mm")
                                nc.tensor.matmul(
                                    out=mm2[:], lhsT=W[name, "wh", 1][:],
                                    rhs=y1[:],
                                    start=True, stop=True,
                                )
                                mm2s.append(mm2)
                            y2s = _emit_elu_stage(
                                nc, apool, ypool,
                                [(mm2[:], W[name, "bh", 1][:]) for mm2 in mm2s],
                                t_mode,
                            )
                            for b, y2 in zip(bs, y2s):
                                # out row b accumulates into partition b of po
                                nc.tensor.matmul(
                                    out=po[:], lhsT=W[name, "we", b][:],
                                    rhs=y2[:],
                                    start=(b == 0), stop=(b == B - 1),
                                    skip_group_check=True,
                                )
                        stage = opool.tile([B, TW], FP, tag="ost")
                        nc.scalar.copy(out=stage[:], in_=po[:])
                        nc.sync.dma_start(
                            out=out_d[name][:, ti * TW : (ti + 1) * TW], in_=stage[:]
                        )
    nc.compile()
    return nc


def _get_nc(reps=1, t_mode="alt"):
    key = (reps, t_mode)
    if key not in _CACHE:
        _CACHE[key] = _build(reps, t_mode)
    return _CACHE[key]


def kernel(**inputs):
    nc = _get_nc()
    f32 = np.float32

    def arr(x, dt=f32):
        return np.ascontiguousarray(np.asarray(x), dtype=dt)

    base = {
        "z": arr(inputs["z"]),
        "template_node_repr": arr(inputs["template_node_repr"]),
    }
    for name in HEADS:
        k = HEAD_K[name]
        base[f"{name}_Win"] = arr(inputs[f"{name}_Win"])
        base[f"{name}_bin"] = arr(inputs[f"{name}_bin"]).reshape(H, 1)
        base[f"{name}_Whid"] = arr(inputs[f"{name}_Whid"])
        base[f"{name}_bhid"] = arr(inputs[f"{name}_bhid"]).reshape(2, H, 1)
        base[f"{name}_Wout"] = arr(inputs[f"{name}_Wout"]).reshape(H, 1)

    bond_idx = arr(inputs["bond_index"], np.int32)
    angle_idx = arr(inputs["angle_index"], np.int32)
    tors_idx = arr(inputs["torsion_index"], np.int32)

    in_maps = []
    for c in range(NCORES):
        m = dict(base)
        m["bond_index"] = np.ascontiguousarray(bond_idx[c * NBc : (c + 1) * NBc])
        m["angle_index"] = np.ascontiguousarray(angle_idx[c * NAc : (c + 1) * NAc])
        m["torsion_index"] = np.ascontiguousarray(tors_idx[c * NDc : (c + 1) * NDc])
        in_maps.append(m)

    res = run_bass_kernel_spmd(nc, in_maps, core_ids=list(range(NCORES)))
    outs = res.results

    def cat(name):
        full = np.concatenate([outs[c][f"{name}_out"] for c in range(NCORES)], axis=1)
        bout = float(np.asarray(inputs[f"{name}_bout"]).reshape(-1)[0])
        return (full + f32(bout)).astype(f32)

    return cat("bond"), cat("angle"), cat("dcos"), cat("dsin")
